# revision 41
# baseline (speedup 1.0000x reference)
"""Multi-head attention (embed 1024, 16 heads x 64) on 8 TRN2 NeuronCores.

Sharding: tensor-parallel over heads — each core owns 2 heads end-to-end
(qkv projection columns + attention), then per-(batch, head) AllToAlls
redistribute the per-head attention outputs so each core computes the
out-projection for its 256-token slice of each batch.

Compute is bf16 on the TensorEngine (fp32 PSUM accumulation). The engines
execute statically-ordered instruction streams, so emission order is
software-pipelined: projection chunks and out-projection slices are woven
INSIDE attention iterations, and each iteration's softmax normalization
(reciprocal + broadcast + multiply) is deferred by one iteration so the
slow one-partition reciprocal never blocks the PE or DVE streams.

v3 layout — PE-array column tiling across tq-halves:
  - an iteration covers one (batch, head) over all 2048 queries: the two
    1024-query HALVES' PV matmuls are column-tiled onto the PE array
    (tq-half 0 -> array cols 0-63 / PSUM partitions 0-63, half 1 -> cols
    64-127) and run CONCURRENTLY (tile_position auto-derives from
    out.base_partition). This halves PV's effective PE time vs the serial
    M=65 form, while keeping the (batch, head) completion order — so the
    per-(batch, head) AllToAlls stagger exactly as in the serial design
    (the last A2A's wire time overlaps the final iteration).
  - scores keep the kt/kt2 row-group trick: the two 512-query quarters of
    a score tile contract on PE rows 0-63 / 64-127 concurrently.
  - the softmax denominator no longer rides a ones-column in V (which
    forced M=65 and blocked col-tiling): the DVE accumulates each half's
    exp tiles (bf16 ping-pong accumulators), then a col-tiled pair of
    M=1 ones-matmuls reduces the 128 partitions into [1, tq] per half.
    The reciprocal still runs 128-wide via the DMA spread (a 1-lane
    [1,1024] reciprocal costs 6.5us on the DVE; spread it is ~200ns).
  - the recip broadcast is a col-tiled K=1 matmul PAIR (both halves in
    one [128, 512] PSUM tile), and ONE DVE multiply normalizes both.
  - the first iteration ramps tq-half 0 ahead of half 1 (half 1's queries
    need x chunks 2-3 projected); its PV runs as un-paired halves until
    both halves stream, then pairing resumes — adjacent emissions with
    disjoint col-groups overlap on the hardware either way.

Tail discipline: everything gated on a PEER (gathers, batch-1 out-proj)
is emitted strictly after the last A2A trigger — under cross-core skew
(observed up to ~16us between SPMD ranks) a peer-gated DMA emitted
earlier stalls the local queues and compounds the skew. Weight matrices
arrive host-pre-transposed so weight DMAs read 2KB-contiguous lines.
"""

import numpy as np
import ml_dtypes

import concourse.bass as bass
import concourse.tile as tile
from concourse import bacc, mybir
from concourse.bass_utils import run_bass_kernel_spmd
from concourse.masks import make_identity

N_CORES = 8
B, S, D = 2, 2048, 1024
T = B * S              # 4096 flattened tokens
HEADS = 16
DH = 64                # head dim
HPC = HEADS // N_CORES  # heads per core = 2
CW = HPC * DH          # per-core qkv width = 128
SCALE = DH ** -0.5
TC = T // N_CORES      # per-core output rows = 512 (256 per batch)
NW = TC // 2           # tokens per batch slice = 256
ET = D // 128          # e partition tiles = 8
F32 = mybir.dt.float32
BF16 = mybir.dt.bfloat16
EXP = mybir.ActivationFunctionType.Exp
BF = ml_dtypes.bfloat16

_CACHED_NC = None


def build():
    nc = bacc.Bacc(
        "TRN2",
        target_bir_lowering=False,
        debug=False,
        num_devices=N_CORES,
    )
    xt_ap = nc.dram_tensor("xt", [D, T], BF16, kind="ExternalInput").ap()
    # w_qkv slices arrive host-transposed as [p, et*c] so the weight DMA
    # reads 2KB-contiguous lines per partition
    wq_ap = nc.dram_tensor("wq", [128, ET * CW], BF16, kind="ExternalInput").ap()
    wk_ap = nc.dram_tensor("wk", [128, ET * CW], BF16, kind="ExternalInput").ap()
    wv_ap = nc.dram_tensor("wv", [128, ET * CW], BF16, kind="ExternalInput").ap()
    bq_ap = nc.dram_tensor("bq", [CW, 1], F32, kind="ExternalInput").ap()
    bk_ap = nc.dram_tensor("bk", [CW, 1], F32, kind="ExternalInput").ap()
    bv_ap = nc.dram_tensor("bv", [CW, 1], F32, kind="ExternalInput").ap()
    wout_ap = nc.dram_tensor("wout", [D, D], BF16, kind="ExternalInput").ap()
    bout_ap = nc.dram_tensor("bout", [128, ET], F32, kind="ExternalInput").ap()
    out_ap = nc.dram_tensor("out", [D, TC], F32, kind="ExternalOutput").ap()

    with tile.TileContext(nc) as tc:
        with (
            tc.tile_pool(name="singles", bufs=1) as singles,
            # 24 bufs: chunk 3's eight tiles ring-reuse chunk 0's slots —
            # batch-0 x is dead once its projections finish in iteration 0
            tc.tile_pool(name="xt", bufs=24) as xt_pool,
            tc.tile_pool(name="vt", bufs=4) as vt_pool,
            # 12 bufs: the iteration-0 ramp (tq-half 0 running ahead of
            # half 1) peaks at 10 live exp tiles; the pool must be strictly
            # deeper than the peak or a new EXP claims a slot whose
            # previous tile's PV consumer is not yet emitted (data race)
            tc.tile_pool(name="exp", bufs=12) as exp_pool,
            tc.tile_pool(name="fo", bufs=4) as fo_pool,
            tc.tile_pool(name="small", bufs=4) as small_pool,
            tc.tile_pool(name="saved", bufs=8) as saved_pool,
            tc.tile_pool(name="mmps", bufs=2, space="PSUM") as mmps,
            tc.tile_pool(name="stps", bufs=2, space="PSUM") as stps,
            tc.tile_pool(name="pvps", bufs=1, space="PSUM") as pvps,
            tc.tile_pool(name="dram", bufs=1, space="DRAM") as dram,
        ):
            # A2A bounce buffers, one pair per (batch, head): shard j holds
            # tokens [j*256,(j+1)*256) of batch b, 64 head-dims per shard.
            dummy_in = dram.tile([N_CORES, 16], BF16, name="dummy_in")
            dummy_out = dram.tile([N_CORES, 16], BF16, name="dummy_out")
            a2a_in = [
                [dram.tile([N_CORES * DH, NW], BF16, name=f"a2a_in{b}_{h}")
                 for h in range(HPC)] for b in range(2)
            ]
            a2a_out = [
                [dram.tile([N_CORES * DH, NW], BF16, name=f"a2a_out{b}_{h}")
                 for h in range(HPC)] for b in range(2)
            ]

            # ---- constants / weights resident in SBUF ----
            # identity first: it has no DMA dependency, so the PE warm-up
            # matmuls below can start while the input DMAs stream
            identb = singles.tile([128, 128], BF16)
            make_identity(nc, identb)
            # ramp the PE p-state while the first x chunk + weights are in
            # flight: the projections otherwise start at 0.65-1.2GHz and
            # burn ~7us extra before the clock reaches full speed
            for _ in range(24):
                warm = mmps.tile([128, 128], F32, tag="mm", name="warm0")
                nc.tensor.matmul(warm, identb, identb)
            w_sb, b_sb = {}, {}
            for name, wap, bap in (
                ("q", wq_ap, bq_ap), ("k", wk_ap, bk_ap), ("v", wv_ap, bv_ap)
            ):
                w_sb[name] = singles.tile(
                    [128, ET, CW], BF16, tag=f"w{name}", name=f"w{name}_sb"
                )
                nc.gpsimd.dma_start(
                    out=w_sb[name],
                    in_=wap.rearrange("p (et c) -> p et c", et=ET),
                )
                b_sb[name] = singles.tile(
                    [CW, 1], F32, tag=f"b{name}", name=f"b{name}_sb"
                )
                nc.gpsimd.dma_start(out=b_sb[name], in_=bap)
            bout_sb = singles.tile([128, ET], F32)
            nc.gpsimd.dma_start(out=bout_sb, in_=bout_ap)
            ones64 = singles.tile([1, DH], BF16)
            nc.vector.memset(ones64, 1.0)
            ones128 = singles.tile([128, 1], BF16)
            nc.vector.memset(ones128, 1.0)
            # dummy collective: absorbs the collective-stream entry barrier +
            # first-trigger latency while the PE ramps. Emitted after the
            # preamble DMAs so the trigger's barrier wait doesn't stall them.
            nc.gpsimd.collective_compute(
                "AllToAll",
                mybir.AluOpType.bypass,
                replica_groups=[list(range(N_CORES))],
                ins=[dummy_in[:, :].opt()],
                outs=[dummy_out[:, :].opt()],
            )
            wout_sb = singles.tile([128, ET, D], BF16, tag="wout")

            # persistent activations
            qt = singles.tile([CW, T], BF16, tag="qt")   # [2h*64, t] transposed Q
            kt = singles.tile([CW, T], BF16, tag="kt")
            # partition-swapped copy of kt: alternating score matmuls load
            # weights into the OTHER PE-array row group, so each LDWEIGHTS
            # pulls ahead of the in-flight matmul instead of serializing
            kt2 = singles.tile([CW, T], BF16, tag="kt2")
            qt2 = singles.tile([CW, T], BF16, tag="qt2")
            # V natural per head, 64-wide tk-tiles (no ones column — the
            # denominator is accumulated on the DVE so PV can col-tile)
            vsb = [
                singles.tile(
                    [128, T // 128, DH], BF16, tag=f"v{h}", name=f"v{h}_sb"
                )
                for h in range(HPC)
            ]
            # denominator accumulators: per tq-half, SEPARATE even-tile and
            # odd-tile chains accumulated in-place. Consecutive DVE ops
            # then never read an address the immediately-preceding op wrote
            # (the DVE's write-ack is pipelined, so a back-to-back
            # same-tile read-after-write can observe stale data — seen as
            # rank-dependent NaN when one half's chain ran alone).
            accs = [
                [singles.tile([128, 1024], BF16, tag=f"acc{tqh}_{eo}",
                              name=f"acc{tqh}_{eo}")
                 for eo in range(2)] for tqh in range(2)
            ]
            # gathered head-features for this core's token rows, per batch
            g_sb = [
                singles.tile([128, ET, NW], BF16, tag=f"g{b}", name=f"g{b}_sb")
                for b in range(2)
            ]

            xt_view = xt_ap.rearrange("(et p) t -> p et t", p=128)

            chunk1024 = {}

            def emit_chunk_dma(tch2, spread=False, spread2=False):
                # 1024-token transfers: the x DMAs are DESCRIPTOR-RATE
                # bound (~42ns per per-partition run), so 1KB runs cap a
                # queue at ~24GB/s — 2KB runs double it. spread=True issues
                # the per-et DMAs from otherwise-idle sequencers (preamble
                # only): the sync sequencer takes ~600ns of descriptor-gen
                # per dma_start. (Only sync/scalar/gpsimd can issue DMAs;
                # mid-attention chunks stay on sync — scalar carries EXPs
                # and gpsimd carries swaps/scatters/collective triggers.
                # spread2 fans across sync/gpsimd, avoiding ScalarE.)
                if spread:
                    engs = [nc.sync, nc.sync, nc.sync, nc.scalar,
                            nc.scalar, nc.scalar, nc.gpsimd, nc.sync]
                elif spread2:
                    engs = [nc.sync, nc.gpsimd, nc.sync, nc.gpsimd,
                            nc.sync, nc.gpsimd, nc.sync, nc.sync]
                else:
                    engs = [nc.sync] * ET
                xs = []
                for et in range(ET):
                    xe = xt_pool.tile([128, 1024], BF16, tag="xt", name="xt_e")
                    # two half-partition DMAs per et tile: a DMA queue
                    # retires ~one 2KB descriptor per 42ns, so one [128,1024]
                    # transfer pins a queue for ~5.4us — the split doubles
                    # queue parallelism and halves the landing time
                    for ph in range(2):
                        engs[et].dma_start(
                            out=xe[ph * 64:(ph + 1) * 64, :],
                            in_=xt_view[ph * 64:(ph + 1) * 64, et,
                                        tch2 * 1024:(tch2 + 1) * 1024],
                        )
                    xs.append(xe)
                chunk1024[tch2] = xs

            def chunk_view(tch, et):
                """512-token projection view into the 1024-token tiles."""
                half = tch % 2
                return chunk1024[tch // 2][et][:, half * 512:(half + 1) * 512]


            def emit_proj_halves(tch, name):
                """One projection (q/k/v) of a 512-token chunk, split into
                two filler closures (4 accumulating matmuls each, PSUM
                accumulator carried across). The scores->EXP pipeline has
                only a 2-tile lookahead (stps double buffer), so a full
                ~1.8us projection piece woven between score tiles drains it
                and stalls ScalarE ~1us; half-pieces keep the bubble short."""
                st = {}

                def half_a():
                    st["pp"] = mmps.tile(
                        [CW, 512], F32, tag="mm", name="pp_proj"
                    )
                    for et in range(ET // 2):
                        nc.tensor.matmul(
                            st["pp"],
                            w_sb[name][:, et, :],
                            chunk_view(tch, et),
                            start=(et == 0),
                            stop=False,
                        )

                def half_b():
                    _finish_proj(tch, name, st["pp"])

                return half_a, half_b

            def emit_proj(tch, name):
                a, b = emit_proj_halves(tch, name)
                a()
                b()

            def _finish_proj(tch, name, pp):
                dest = {"q": qt, "k": kt, "v": None}[name]
                for et in range(ET // 2, ET):
                    nc.tensor.matmul(
                        pp,
                        w_sb[name][:, et, :],
                        chunk_view(tch, et),
                        start=False,
                        stop=(et == ET - 1),
                    )
                if dest is not None:
                    nc.vector.tensor_scalar_add(
                        dest[:, tch * 512:(tch + 1) * 512], pp, b_sb[name]
                    )
                    src_t, dst_t = (kt, kt2) if name == "k" else (qt, qt2)
                    sl = slice(tch * 512, (tch + 1) * 512)
                    # first-chunk swaps dodge the gpsimd queue: the dummy
                    # collective's entry-barrier wait sits there and would
                    # delay qt2/kt2 (gating the first score tiles). ScalarE's
                    # queue is free until the first EXP fires.
                    eng = nc.scalar if tch < 2 else nc.gpsimd
                    eng.dma_start(
                        out=dst_t[DH:2 * DH, sl], in_=src_t[0:DH, sl]
                    )
                    eng.dma_start(
                        out=dst_t[0:DH, sl], in_=src_t[DH:2 * DH, sl]
                    )
                else:
                    vt_tmp = vt_pool.tile([CW, 512], BF16, name="vt_tmp")
                    nc.vector.tensor_scalar_add(vt_tmp, pp, b_sb[name])
                    for tt in range(4):
                        ps2 = mmps.tile([128, 128], BF16, tag="mm", name="ps_vtr")
                        nc.tensor.transpose(
                            ps2, vt_tmp[:, tt * 128:(tt + 1) * 128], identb
                        )
                        ttg = tch * 4 + tt
                        for h in range(HPC):
                            nc.vector.tensor_copy(
                                vsb[h][:, ttg, :],
                                ps2[:, h * DH:(h + 1) * DH],
                            )

            def emit_projs(tch):
                for name in ("q", "k", "v"):
                    emit_proj(tch, name)

            def weave(b, h, fillers, scalar_copy=False):
                """Attention iteration with filler pieces spread between
                tk-tile groups so ScalarE's exp stream never starves while
                the PE works through a filler."""
                A = HeadIter(b, h, scalar_copy=scalar_copy)
                k = len(fillers)
                for i, f in enumerate(fillers):
                    A.advance((i + 1) * 16 // (k + 1))
                    f()
                return A.finish()

            class HeadIter:
                """Resumable attention iteration: ALL 2048 queries of head
                h, batch b, as two 1024-query halves. advance2(hi0, hi1)
                emits score/exp/PV work for tk-tiles up to the per-half
                targets (PV pipelined behind scores); finish() drains,
                reduces the denominators and returns (pvc, recips) for the
                deferred normalization. The two halves' PV matmuls col-tile
                onto array cols 0-63 / 64-127 and overlap whenever their
                emissions are adjacent; the DVE accumulates each half's exp
                tiles for the softmax denominator while the PE works."""

                def __init__(self, b, h, scalar_copy=False):
                    self.b, self.h = b, h
                    self.pv = pvps.tile([128, 1024], F32, name="pv")
                    self.exs = {}
                    self.sc = [0, 0]
                    self.pvd = [0, 0]
                    self.scalar_copy = scalar_copy

                def _scores(self, tqh, tkt):
                    st = stps.tile([128, 1024], F32, tag="st", name="st")
                    po = self.h * DH
                    po2 = DH - po
                    k0 = self.b * S + tkt * 128
                    tq0 = self.b * S + tqh * 1024
                    for nh in range(2):
                        if nh == 0:
                            lhsT = kt[po:po + DH, k0:k0 + 128]
                            rhs_q = qt[po:po + DH,
                                       tq0 + nh * 512:tq0 + (nh + 1) * 512]
                        else:
                            lhsT = kt2[po2:po2 + DH, k0:k0 + 128]
                            rhs_q = qt2[po2:po2 + DH,
                                        tq0 + nh * 512:tq0 + (nh + 1) * 512]
                        nc.tensor.matmul(
                            st[:, nh * 512:(nh + 1) * 512], lhsT, rhs_q
                        )
                    ex = exp_pool.tile([128, 1024], BF16, name="ex")
                    nc.scalar.activation(ex, st, EXP)
                    # denominator partial: even/odd chains, in-place on the
                    # DVE (bf16 tensor_tensor runs 2 elem/cycle/lane)
                    eo = tkt % 2
                    if tkt < 2:
                        nc.vector.tensor_copy(accs[tqh][eo], ex)
                    else:
                        nc.vector.tensor_add(
                            accs[tqh][eo], accs[tqh][eo], ex
                        )
                    self.exs[(tqh, tkt)] = ex

                def _pv_half(self, tqh, tkt):
                    # two tk-tiles per PV group, ordered so consecutive
                    # matmuls hit the same psum bank; the half's output
                    # col-tiles at (0, tqh*64), so the two halves' groups
                    # run concurrently whenever interleaved
                    e0 = self.exs.pop((tqh, tkt))
                    e1 = self.exs.pop((tqh, tkt + 1))
                    for nh in range(2):
                        for tt, ex in ((tkt, e0), (tkt + 1, e1)):
                            nc.tensor.matmul(
                                self.pv[tqh * DH:(tqh + 1) * DH,
                                        nh * 512:(nh + 1) * 512],
                                vsb[self.h][:, self.b * (T // 256) + tt, :],
                                ex[:, nh * 512:(nh + 1) * 512],
                                start=(tt == 0),
                                stop=(tt == 15),
                            )

                def _pv_pair(self, tkt):
                    # both halves' groups interleaved per matmul so every
                    # adjacent (col 0-63, col 64-127) pair overlaps on the
                    # PE array — emitting a half's 4 matmuls as a block
                    # serializes them (same col group)
                    exq = {
                        tqh: (self.exs.pop((tqh, tkt)),
                              self.exs.pop((tqh, tkt + 1)))
                        for tqh in range(2)
                    }
                    for nh in range(2):
                        for i, tt in enumerate((tkt, tkt + 1)):
                            for tqh in range(2):
                                nc.tensor.matmul(
                                    self.pv[tqh * DH:(tqh + 1) * DH,
                                            nh * 512:(nh + 1) * 512],
                                    vsb[self.h][:, self.b * (T // 256) + tt, :],
                                    exq[tqh][i][:, nh * 512:(nh + 1) * 512],
                                    start=(tt == 0),
                                    stop=(tt == 15),
                                )

                def _drain_pv(self):
                    # PV trails the scores by one extra tile so its first
                    # matmul consumes an EXP that finished >=2 tiles ago —
                    # it never stalls on a fresh ScalarE completion.
                    while True:
                        can = [self.pvd[q] + 1 < self.sc[q] - 2
                               for q in range(2)]
                        if can[0] and can[1] and self.pvd[0] == self.pvd[1]:
                            self._pv_pair(self.pvd[0])
                            self.pvd[0] += 2
                            self.pvd[1] += 2
                        elif can[0] and (self.pvd[0] <= self.pvd[1]
                                         or not can[1]):
                            self._pv_half(0, self.pvd[0])
                            self.pvd[0] += 2
                        elif can[1]:
                            self._pv_half(1, self.pvd[1])
                            self.pvd[1] += 2
                        else:
                            break

                def advance2(self, hi0, hi1):
                    while self.sc[0] < hi0 or self.sc[1] < hi1:
                        for tqh, tgt in ((0, hi0), (1, hi1)):
                            if self.sc[tqh] < tgt:
                                self._scores(tqh, self.sc[tqh])
                                self.sc[tqh] += 1
                        self._drain_pv()

                def advance(self, tk_hi):
                    self.advance2(tk_hi, tk_hi)

                def finish(self):
                    self.advance2(16, 16)
                    while self.pvd[0] < 16 and self.pvd[0] == self.pvd[1]:
                        self._pv_pair(self.pvd[0])
                        self.pvd[0] += 2
                        self.pvd[1] += 2
                    for tqh in range(2):
                        while self.pvd[tqh] < 16:
                            self._pv_half(tqh, self.pvd[tqh])
                            self.pvd[tqh] += 2
                    # denominator partition-reduce: col-tiled M=1 pairs per
                    # 512-quarter — half tqh's sum lands on PSUM partition
                    # tqh*32, accumulating the even and odd chains.
                    d_ps = []
                    for nh in range(2):
                        dp = mmps.tile([128, 512], F32, tag="mm", name="d_ps")
                        for eo in range(2):
                            for tqh in range(2):
                                nc.tensor.matmul(
                                    dp[tqh * 32:tqh * 32 + 1, :],
                                    ones128,
                                    accs[tqh][eo][:, nh * 512:(nh + 1) * 512],
                                    start=(eo == 0),
                                    stop=(eo == 1),
                                )
                        d_ps.append(dp)
                    # numerator leaves PSUM in ONE bf16 copy (frees the PV
                    # accumulator). In the tail the copy goes to the idle
                    # ScalarE so the DVE is free for what follows.
                    pvc = fo_pool.tile(
                        [128, 1024], BF16, tag="pvc", name="pvc"
                    )
                    if self.scalar_copy:
                        nc.scalar.activation(
                            pvc, self.pv, mybir.ActivationFunctionType.Copy
                        )
                    else:
                        nc.vector.tensor_copy(pvc, self.pv)
                    # DMA cannot read PSUM: bounce the two denominator rows
                    # to SBUF first (one [33, 512] DVE copy per half costs
                    # the same 512 free-dim cycles as a single row)
                    d_sb = small_pool.tile([33, 1024], F32, tag="dsb",
                                           name="d_sb")
                    for nh in range(2):
                        nc.vector.tensor_copy(
                            d_sb[:, nh * 512:(nh + 1) * 512], d_ps[nh][0:33, :]
                        )
                    # denominator rows are DMA-spread across 128 partitions
                    # so the reciprocal runs 128-wide (a 1-lane [1,1024]
                    # reciprocal costs 6.5us on the DVE). Both hops split
                    # across two queues (gpsimd + sync).
                    recips = []
                    for tqh in range(2):
                        dn = small_pool.tile(
                            [128, 8], F32, tag="dn128", name="dn128"
                        )
                        nc.gpsimd.dma_start(
                            out=dn[0:64, :],
                            in_=d_sb[tqh * 32:tqh * 32 + 1, 0:512],
                        )
                        nc.sync.dma_start(
                            out=dn[64:128, :],
                            in_=d_sb[tqh * 32:tqh * 32 + 1, 512:1024],
                        )
                        r128 = small_pool.tile(
                            [128, 8], BF16, tag="r128", name="r128"
                        )
                        with nc.allow_low_precision(
                            reason="softmax denom reciprocal feeds bf16 bcast"
                        ):
                            nc.vector.reciprocal(r128, dn)
                        rrow = small_pool.tile([1, 1024], BF16, name="recip")
                        nc.gpsimd.dma_start(
                            out=rrow[:, 0:512], in_=r128[0:64, :]
                        )
                        nc.sync.dma_start(
                            out=rrow[:, 512:1024], in_=r128[64:128, :]
                        )
                        recips.append(rrow)
                    return pvc, recips

            def emit_finish(b, h, pvc, recips):
                """Deferred normalization + scatter into the A2A input.
                The recip broadcast is a col-tiled K=1 matmul pair (both
                tq-halves in one [128, 512] PSUM tile); ONE DVE multiply
                then normalizes both halves at once."""
                a2a_view = a2a_in[b][h][:, :].rearrange("(j p) t -> p j t", p=DH)
                for nh in range(2):
                    bc = mmps.tile([128, 512], F32, tag="mm", name="bc")
                    for tqh in range(2):
                        nc.tensor.matmul(
                            bc[tqh * DH:(tqh + 1) * DH, :],
                            ones64,
                            recips[tqh][:, nh * 512:(nh + 1) * 512],
                        )
                    fo = fo_pool.tile([128, 512], BF16, tag="fo", name="fo")
                    nc.vector.tensor_mul(
                        fo, pvc[:, nh * 512:(nh + 1) * 512], bc
                    )
                    # per-shard queue split: this scatter gates the A2A
                    # trigger directly. Half tqh's rows cover global tq
                    # [tqh*1024 + nh*512, +512) -> shards tqh*4 + 2*nh + j.
                    for tqh in range(2):
                        fo_j = fo[tqh * DH:(tqh + 1) * DH, :].rearrange(
                            "p (j t) -> p j t", j=2
                        )
                        for j, eng in ((0, nc.gpsimd), (1, nc.sync)):
                            eng.dma_start(
                                out=a2a_view[:, tqh * 4 + 2 * nh + j:
                                             tqh * 4 + 2 * nh + j + 1, :],
                                in_=fo_j[:, j:j + 1, :],
                            )

            def emit_a2a(b, h):
                nc.gpsimd.collective_compute(
                    "AllToAll",
                    mybir.AluOpType.bypass,
                    replica_groups=[list(range(N_CORES))],
                    ins=[a2a_in[b][h][:, :].opt()],
                    outs=[a2a_out[b][h][:, :].opt()],
                )

            def emit_gather(b, h, split=False):
                # rank r's shard lands at rows r*64..(r+1)*64 → head h's dims
                # are partitions h*64..h*64+64 of g_sb[b][:, r, :]. Emitted
                # just before the first consumer: this DMA waits on the
                # collective, and the sync engine issues triggers in order —
                # an early emission would stall every later DMA behind it.
                src = a2a_out[b][h][:, :].rearrange("(r p) t -> p r t", p=DH)
                if not split:
                    nc.sync.dma_start(
                        out=g_sb[b][h * DH:(h + 1) * DH, :, :], in_=src
                    )
                else:
                    # tail gathers sit on the mesh-end -> out-proj critical
                    # path: halve their drain time across two queues
                    # (ScalarE's queue is idle once the EXPs are done)
                    for rh, eng in ((0, nc.sync), (1, nc.scalar)):
                        eng.dma_start(
                            out=g_sb[b][h * DH:(h + 1) * DH,
                                        rh * 4:(rh + 1) * 4, :],
                            in_=src[:, rh * 4:(rh + 1) * 4, :],
                        )

            def emit_outproj(b, ot, dma_eng=None):
                """Out-projection columns [ot*128,(ot+1)*128) for batch b."""
                pp = mmps.tile([128, NW], F32, tag="mm", name="pp_out")
                for ht in range(ET):
                    nc.tensor.matmul(
                        pp,
                        wout_sb[:, ht, ot * 128:(ot + 1) * 128],
                        g_sb[b][:, ht, :],
                        start=(ht == 0),
                        stop=(ht == ET - 1),
                    )
                ob = vt_pool.tile([128, NW], F32, tag="ob", name="ob")
                nc.vector.tensor_scalar_add(ob, pp, bout_sb[:, ot:ot + 1])
                # output stays column-major [o, t]; the host transposes
                # during unshard — keeps 32 PE transposes + copies out of
                # the clock-throttled tail
                (dma_eng or nc.sync).dma_start(
                    out=out_ap[ot * 128:(ot + 1) * 128, b * NW:(b + 1) * NW],
                    in_=ob,
                )

            def emit_outproj_halves(b, ot):
                """emit_outproj split into two filler closures (see
                emit_proj_halves for why)."""
                st = {}

                def half_a():
                    st["pp"] = mmps.tile([128, NW], F32, tag="mm", name="pp_out")
                    for ht in range(ET // 2):
                        nc.tensor.matmul(
                            st["pp"],
                            wout_sb[:, ht, ot * 128:(ot + 1) * 128],
                            g_sb[b][:, ht, :],
                            start=(ht == 0),
                            stop=False,
                        )

                def half_b():
                    pp = st["pp"]
                    for ht in range(ET // 2, ET):
                        nc.tensor.matmul(
                            pp,
                            wout_sb[:, ht, ot * 128:(ot + 1) * 128],
                            g_sb[b][:, ht, :],
                            start=False,
                            stop=(ht == ET - 1),
                        )
                    ob = vt_pool.tile([128, NW], F32, tag="ob", name="ob")
                    nc.vector.tensor_scalar_add(ob, pp, bout_sb[:, ot:ot + 1])
                    nc.sync.dma_start(
                        out=out_ap[ot * 128:(ot + 1) * 128,
                                   b * NW:(b + 1) * NW],
                        in_=ob,
                    )

                return half_a, half_b

            def emit_outproj_p1(b, ot):
                """Lower contraction half of an out-proj column block
                (head-0 features, partitions 0-63 — gathered by the EARLY
                per-batch A2A): real PE fill during the last A2A wait, so
                the clock stays up and only the upper half remains after
                the final gather. Partial saved to SBUF so PSUM rotates."""
                pp = mmps.tile([128, NW], F32, tag="mm", name="pp_h1")
                for ht in range(ET):
                    nc.tensor.matmul(
                        pp,
                        wout_sb[0:64, ht, ot * 128:(ot + 1) * 128],
                        g_sb[b][0:64, ht, :],
                        start=(ht == 0),
                        stop=(ht == ET - 1),
                    )
                sv = saved_pool.tile([128, NW], F32, tag="sv", name="sv")
                nc.vector.tensor_copy(sv, pp)
                return sv

            def emit_outproj_p2(b, ot, sv, dma_eng=None):
                pp = mmps.tile([128, NW], F32, tag="mm", name="pp_h2")
                for ht in range(ET):
                    nc.tensor.matmul(
                        pp,
                        wout_sb[64:128, ht, ot * 128:(ot + 1) * 128],
                        g_sb[b][64:128, ht, :],
                        start=(ht == 0),
                        stop=(ht == ET - 1),
                    )
                ob = vt_pool.tile([128, NW], F32, tag="ob", name="ob")
                # ob = (upper_half + bias) + saved_lower_half, one DVE pass
                nc.vector.scalar_tensor_tensor(
                    ob, pp, bout_sb[:, ot:ot + 1], sv,
                    mybir.AluOpType.add, mybir.AluOpType.add,
                )
                # half-partition DMA split: these are the last transfers
                # before kernel close — two 64-descriptor DMAs drain on two
                # queues in half the time of one 128-descriptor DMA
                for ph in range(2):
                    (dma_eng or nc.sync).dma_start(
                        out=out_ap[ot * 128 + ph * 64:ot * 128 + (ph + 1) * 64,
                                   b * NW:(b + 1) * NW],
                        in_=ob[ph * 64:(ph + 1) * 64, :],
                    )

            # ---- emission schedule: software-pipelined. fin(i) lands one
            # attention iteration after iter(i) so the reciprocal is ready
            # before its broadcast matmul enters the PE stream. Iterations
            # are (batch, head) covering both tq-halves. ----
            pend = {}

            # iteration (0, h0): tq-half 0 ramps first (its queries need
            # only chunks 0-1); half 1 starts once chunks 2-3 project.
            # Both token chunks of batch 0 stream from the preamble — the
            # tq-half-1 queries (proj chunks 2-3) gate on chunk 1, and a
            # late chunk 1 starves the EXP stream for ~10us mid-ramp.
            emit_chunk_dma(0, spread=True)
            emit_chunk_dma(1, spread2=True)
            emit_proj(0, "q")
            emit_proj(0, "k")
            emit_proj(1, "q")
            emit_proj(0, "v")
            I0 = HeadIter(0, 0)
            I0.advance2(4, 0)
            emit_proj(1, "k")
            emit_proj(1, "v")
            I0.advance2(8, 0)
            emit_proj(2, "q")
            emit_proj(3, "q")
            # half 1 may start (q2/q3 emitted) but half 0 must NOT pass
            # tile 8 until proj(2,'k') is emitted — a score emitted before
            # its kt chunk's projection reads stale SBUF (no dependency
            # exists yet for a region whose write comes later)
            I0.advance2(8, 2)
            emit_proj(2, "k")
            emit_proj(2, "v")
            I0.advance2(12, 6)
            emit_chunk_dma(2)
            emit_proj(3, "k")
            emit_proj(3, "v")
            pend[0] = I0.finish()
            # remaining projection / out-proj pieces are woven INSIDE later
            # iterations as HALF-pieces: a block between two iterations
            # stalls the EXP stream for its whole duration. Batch-1 q must
            # fully project before iteration (1, h0); its k/v chunks land
            # just ahead of the score/PV tiles that consume them.
            p4q = emit_proj_halves(4, "q")
            p4k = emit_proj_halves(4, "k")
            p4v = emit_proj_halves(4, "v")
            p5q = emit_proj_halves(5, "q")
            p6q = emit_proj_halves(6, "q")
            p7q = emit_proj_halves(7, "q")
            pend[1] = weave(0, 1, [
                lambda: emit_chunk_dma(3, spread2=True),
                p4q[0], p4q[1],
                lambda: emit_finish(0, 0, *pend[0]),
                p4k[0], p4k[1],
                p5q[0], p5q[1],
                p6q[0], p6q[1],
                p7q[0], p7q[1],
                p4v[0], p4v[1],
            ])
            emit_a2a(0, 0)
            # 2MB weight load split across two queues: 1024 descriptors
            # on one queue is ~43us of drain — too close to its first
            # consumer under co-tenant DMA pressure
            wout_view = wout_ap.rearrange("(ht p) o -> p ht o", p=128)
            p5k = emit_proj_halves(5, "k")
            p5v = emit_proj_halves(5, "v")
            p6k = emit_proj_halves(6, "k")
            p6v = emit_proj_halves(6, "v")
            p7k = emit_proj_halves(7, "k")
            p7v = emit_proj_halves(7, "v")
            pend[2] = weave(1, 0, [
                p5k[0], p5k[1],
                lambda: emit_finish(0, 1, *pend[1]),
                p5v[0], p5v[1],
                p6k[0], p6k[1],
                p6v[0], p6v[1],
                p7k[0], p7k[1],
                lambda: (
                    nc.gpsimd.dma_start(
                        out=wout_sb[:, 0:4, :], in_=wout_view[:, 0:4, :]),
                    nc.sync.dma_start(
                        out=wout_sb[:, 4:8, :], in_=wout_view[:, 4:8, :]),
                ),
                p7v[0], p7v[1],
            ])
            emit_a2a(0, 1)
            # iteration (1, h1), interleaved with batch-0 out-projection;
            # a2a(1,0) fires mid-iteration right after (1, h0)'s finish, so
            # its wire time overlaps the remaining attention.
            op0 = [emit_outproj_halves(0, ot) for ot in range(5)]
            pend[3] = weave(1, 1, [
                lambda: (emit_finish(1, 0, *pend[2]), emit_a2a(1, 0)),
                lambda: (emit_gather(0, 0), emit_gather(0, 1)),
                op0[0][0], op0[0][1],
                op0[1][0], op0[1][1],
                op0[2][0], op0[2][1],
                op0[3][0], op0[3][1],
                op0[4][0], op0[4][1],
            ], scalar_copy=True)
            # keep the PE busy through the reciprocal DMA round-trip that
            # gates the final normalization: an idle PE drops to 1.2GHz and
            # everything after would run at half rate.
            emit_outproj(0, 5)
            for _ in range(16):
                warm = mmps.tile([128, NW], F32, tag="mm", name="warm")
                nc.tensor.matmul(warm, identb, wout_sb[:, 0, 0:NW])
            emit_finish(1, 1, *pend[3])
            emit_a2a(1, 1)
            # fill the exposed A2A wait: remaining batch-0 columns, then
            # the lower contraction halves of batch-1's out-projections
            # (their head-0 inputs arrived with the earlier a2a(1,0))
            emit_outproj(0, 6)
            emit_outproj(0, 7)
            emit_gather(1, 0, split=True)
            svs = [emit_outproj_p1(1, ot) for ot in range(ET)]
            emit_gather(1, 1, split=True)
            # bridge the window between the last local matmul and the
            # final gather landing (up to ~25us under cross-core skew):
            # ANY idle resets the PE p-state to 1.2GHz, which would double
            # the cost of the 64 upper-half matmuls that follow
            for _ in range(36):
                warm = mmps.tile([128, NW], F32, tag="mm", name="warm")
                nc.tensor.matmul(warm, identb, wout_sb[:, 0, 0:NW])
            # fan the final output DMAs across three sequencers: the
            # batch-1 outputs are 1MB and only become legal after the last
            # gather — serial descriptor-gen on one sequencer would add ~4us
            tail_engs = [nc.sync, nc.scalar, nc.gpsimd]
            for ot in range(ET):
                emit_outproj_p2(1, ot, svs[ot], dma_eng=tail_engs[ot % 3])

    nc.compile()
    return nc


def shard_inputs(x, w_qkv, b_qkv, w_out, b_out):
    """Split full inputs into the 8 per-core input maps (bf16 compute).
    x is transposed host-side so projections need no on-device transpose."""
    x2d = np.asarray(x, dtype=np.float32).reshape(T, D)
    xt = np.ascontiguousarray(x2d.T.astype(BF))  # [D, T]
    w_qkv = np.asarray(w_qkv, dtype=np.float32)
    b_qkv = np.asarray(b_qkv, dtype=np.float32)
    w_out = np.ascontiguousarray(np.asarray(w_out, dtype=np.float32).astype(BF))
    b_out = np.asarray(b_out, dtype=np.float32)
    bout_r = np.ascontiguousarray(b_out.reshape(ET, 128).T)  # [p, ot]
    def rearr(w):
        # [D, CW] -> [128, ET*CW]: row p holds the ET contraction tiles
        # back-to-back so the device-side DMA is line-contiguous
        return np.ascontiguousarray(
            w.reshape(ET, 128, CW).transpose(1, 0, 2).reshape(128, ET * CW)
        )

    in_maps = []
    for i in range(N_CORES):
        c0 = i * CW
        wq = rearr(w_qkv[:, c0:c0 + CW] * SCALE).astype(BF)
        wk = rearr(w_qkv[:, D + c0:D + c0 + CW]).astype(BF)
        wv = rearr(w_qkv[:, 2 * D + c0:2 * D + c0 + CW]).astype(BF)
        bq = (b_qkv[c0:c0 + CW] * SCALE).reshape(CW, 1)
        bk = b_qkv[D + c0:D + c0 + CW].reshape(CW, 1)
        bv = b_qkv[2 * D + c0:2 * D + c0 + CW].reshape(CW, 1)
        in_maps.append({
            "xt": xt,
            "wq": wq, "wk": wk, "wv": wv,
            "bq": np.ascontiguousarray(bq),
            "bk": np.ascontiguousarray(bk),
            "bv": np.ascontiguousarray(bv),
            "wout": w_out,
            "bout": bout_r,
        })
    return in_maps


def get_nc():
    global _CACHED_NC
    if _CACHED_NC is None:
        _CACHED_NC = build()
    return _CACHED_NC


def run(in_maps, trace=False, **kw):
    nc = get_nc()
    return run_bass_kernel_spmd(
        nc, in_maps, core_ids=list(range(N_CORES)), trace=trace, **kw
    )


def assemble(results):
    """Each core returns [1024, 512] column-major: cols 0..255 = its
    256-token slice of batch 0, cols 256..511 = its slice of batch 1;
    transposed to row-major here during unshard."""
    out = np.empty((T, D), dtype=np.float32)
    for i, r in enumerate(results):
        o = r["out"]
        out[i * NW:(i + 1) * NW] = o[:, :NW].T
        out[S + i * NW:S + (i + 1) * NW] = o[:, NW:].T
    return out.reshape(B, S, D)


def kernel(x, w_qkv, b_qkv, w_out, b_out):
    in_maps = shard_inputs(x, w_qkv, b_qkv, w_out, b_out)
    res = run(in_maps, trace=False)
    return assemble(res.results)
